# revision 12
# baseline (speedup 1.0000x reference)
"""Trainium2 Bass kernel for nn_Block_9938554323537.

Strategy
--------
Data-parallel over batch B=16 across 8 NeuronCores (2 batch elements each,
no collectives).  The block is algebraically collapsed so the device only
runs 256-wide matmuls on raw x plus per-token scalar corrections:

  * LayerNorm folds into per-token (mu, r) affine corrections applied after
    matmuls on raw x (weights pre-scaled by the norm gain on the host).
  * ||q||^2, ||k||^2 come from Gram matrices M_q = wq'^T wq' (256x256), so q/k
    (B,N,2048) are never materialized.
  * The heavy wp (2048x2048) and wf projections collapse into
    W1 = wf @ wp (256x2048, host) and per-batch Bk = (W1*G) @ wk' (256x256,
    device), plus the constant Bq = wf @ wq' (256x256, host).
  * attn = c1q*(x@Bq^T) + c2q*bq1 + c1k*(x@Bk^T) + c2k*bk1 (+x) with
    per-token scalars c1*, c2* — fused scalar_tensor_tensor chains.
  * MLP runs in feature-major layout (PE transposes of standardized z2) so
    gelu+bias fuse into one ACT op per h-chunk.

Device MACs drop ~7.6x vs the naive implementation (19.7G -> 2.6G per batch
element).  Everything except the residual path runs in bf16 on the PE
(fp32 PSUM accumulation); measured end-to-end rel-l2 error ~4e-4.
"""

import os
import numpy as np
import ml_dtypes

import concourse.bass as bass
import concourse.mybir as mybir
import concourse.tile as tile
from concourse import bacc
from concourse.bass_utils import run_bass_kernel_spmd
from concourse.masks import make_identity

BF16 = ml_dtypes.bfloat16
F32 = np.float32

# problem shapes (hardcoded per contract)
B, N, D = 16, 3137, 256
DI, DH = 2048, 1024
NC = 8            # cores
NB = B // NC      # batch elems per core
P = 128
T = (N + P - 1) // P          # 25 token tiles per batch elem
TLAST = N - (T - 1) * P       # 65
MACRO = 512
NMAC = (N + MACRO - 1) // MACRO   # 7
LN_EPS = 1e-5
L2_EPS = 1e-12

AF = mybir.ActivationFunctionType
ALU = mybir.AluOpType
dt = mybir.dt

# ST slot indices (per-token stats, [128, T, NSLOT] f32)
MU, VAR, R, A1Q, A1K, XM1Q, XM1K, XPQ, XPK, XVG, IQ, IK, C1Q, C2Q, C1K, C2K = \
    range(16)
NSLOT = 16


def _build_program(consts):
    """Build the per-core SPMD Tile program.  `consts` carries the host
    scalars baked as immediates."""
    s1q, s1k, pq1, pk1, vg1, cq0, ck0, c0u = (
        consts["s1q"], consts["s1k"], consts["pq1"], consts["pk1"],
        consts["vg1"], consts["cq0"], consts["ck0"], consts["c0u"])
    nz_pq, nz_pk, nz_u = consts["nz_pq"], consts["nz_pk"], consts["nz_u"]

    phases = int(os.environ.get("KERNEL_PHASES", "3"))
    nc = bacc.Bacc("TRN2", target_bir_lowering=False, debug=False,
                   num_devices=NC)

    # ---- DRAM I/O ----
    xtok = nc.dram_tensor("xtok", [NB, N, D], dt.float32, kind="ExternalInput").ap()
    xT = nc.dram_tensor("xT", [NB, D, N], dt.bfloat16, kind="ExternalInput").ap()
    s1_d = nc.dram_tensor("s1", [D, 2 * D], dt.bfloat16, kind="ExternalInput").ap()
    s2_d = nc.dram_tensor("s2", [D, 8], dt.bfloat16, kind="ExternalInput").ap()
    bqt_d = nc.dram_tensor("bqt", [D, D], dt.bfloat16, kind="ExternalInput").ap()
    w1t_d = nc.dram_tensor("w1t", [DI, D], dt.bfloat16, kind="ExternalInput").ap()
    wqt_d = nc.dram_tensor("wqt", [D, DI], dt.bfloat16, kind="ExternalInput").ap()
    wke_d = nc.dram_tensor("wke", [DI, D], dt.bfloat16, kind="ExternalInput").ap()
    m1t_d = nc.dram_tensor("m1t", [D, DH], dt.bfloat16, kind="ExternalInput").ap()
    m2t_d = nc.dram_tensor("m2t", [DH, D], dt.bfloat16, kind="ExternalInput").ap()
    b1e_d = nc.dram_tensor("b1e", [DH], dt.float32, kind="ExternalInput").ap()
    b2e_d = nc.dram_tensor("b2e", [D], dt.float32, kind="ExternalInput").ap()
    bq1_d = nc.dram_tensor("bq1", [D], dt.float32, kind="ExternalInput").ap()
    g2_d = nc.dram_tensor("g2v", [D], dt.float32, kind="ExternalInput").ap()
    b2n_d = nc.dram_tensor("b2n", [D], dt.float32, kind="ExternalInput").ap()
    y = nc.dram_tensor("y", [NB, N, D], dt.float32, kind="ExternalOutput").ap()

    def bcast_ap(ap1d, parts):
        return bass.AP(tensor=ap1d.tensor, offset=ap1d.offset,
                       ap=[[0, parts]] + list(ap1d.ap))

    with tile.TileContext(nc) as tc:
        with (
            tc.tile_pool(name="singles", bufs=1) as singles,
            tc.tile_pool(name="xtokp", bufs=2) as xtokp,
            tc.tile_pool(name="xtp", bufs=2) as xtp,
            tc.tile_pool(name="stp", bufs=2) as stp,
            tc.tile_pool(name="t25", bufs=8) as t25,
            tc.tile_pool(name="work", bufs=8) as work,
            tc.tile_pool(name="outp", bufs=3) as outp,
            tc.tile_pool(name="mvp", bufs=6) as mvp,
            tc.tile_pool(name="smallp", bufs=8) as smallp,
            tc.tile_pool(name="w1gp", bufs=2) as w1gp,
            tc.tile_pool(name="bktp", bufs=2) as bktp,
            tc.tile_pool(name="z2p", bufs=6) as z2p,
            tc.tile_pool(name="z2tp", bufs=2) as z2tp,
            tc.tile_pool(name="hdnp", bufs=2) as hdnp,
            tc.tile_pool(name="bodyp", bufs=2) as bodyp,
            tc.tile_pool(name="psum", bufs=1, space="PSUM") as psum,
        ):
            # ---- constants / weights in SBUF ----
            ident = singles.tile([P, P], dt.bfloat16, tag="ident")
            make_identity(nc, ident)
            eps_t = singles.tile([P, 1], dt.float32, tag="eps")
            nc.vector.memset(eps_t, LN_EPS)
            ones_col_f = singles.tile([P, 1], dt.float32, tag="ocf")
            nc.vector.memset(ones_col_f, 1.0)
            ones_row_f = singles.tile([1, P], dt.float32, tag="orf")
            nc.vector.memset(ones_row_f, 1.0)
            ones_col_b = singles.tile([P, 1], dt.bfloat16, tag="ocb")
            nc.vector.memset(ones_col_b, 1.0)
            ones_row_b = singles.tile([1, P], dt.bfloat16, tag="orb")
            nc.vector.memset(ones_row_b, 1.0)

            s1_s = singles.tile([P, 2, 2 * D], dt.bfloat16, tag="s1")
            nc.sync.dma_start(s1_s, s1_d.rearrange("(s p) n -> p s n", p=P))
            s2_s = singles.tile([P, 2, 8], dt.bfloat16, tag="s2")
            nc.sync.dma_start(s2_s, s2_d.rearrange("(s p) n -> p s n", p=P))
            bqt_s = singles.tile([P, 2, D], dt.bfloat16, tag="bqt")
            nc.sync.dma_start(bqt_s, bqt_d.rearrange("(s p) n -> p s n", p=P))
            w1t_s = singles.tile([P, DI // P, D], dt.bfloat16, tag="w1t")
            nc.sync.dma_start(w1t_s, w1t_d.rearrange("(k p) n -> p k n", p=P))
            wqt_s = singles.tile([P, 2, DI], dt.bfloat16, tag="wqt")
            nc.sync.dma_start(wqt_s, wqt_d.rearrange("(s p) n -> p s n", p=P))
            wke_s = singles.tile([P, DI // P, D], dt.bfloat16, tag="wke")
            nc.sync.dma_start(wke_s, wke_d.rearrange("(k p) n -> p k n", p=P))
            m1t_s = singles.tile([P, 2, DH], dt.bfloat16, tag="m1t")
            nc.sync.dma_start(m1t_s, m1t_d.rearrange("(s p) n -> p s n", p=P))
            m2t_s = singles.tile([P, DH // P, D], dt.bfloat16, tag="m2t")
            nc.sync.dma_start(m2t_s, m2t_d.rearrange("(k p) n -> p k n", p=P))
            b1e_s = singles.tile([P, DH // P], dt.float32, tag="b1e")
            nc.sync.dma_start(b1e_s, b1e_d.rearrange("(h p) -> p h", p=P))
            b2e_s = singles.tile([P, 2], dt.float32, tag="b2e")
            nc.sync.dma_start(b2e_s, b2e_d.rearrange("(h p) -> p h", p=P))
            bq1_t = singles.tile([P, D], dt.float32, tag="bq1")
            nc.gpsimd.dma_start(out=bq1_t, in_=bcast_ap(bq1_d, P))
            g2row = singles.tile([1, D], dt.float32, tag="g2r")
            nc.gpsimd.dma_start(out=g2row, in_=bcast_ap(g2_d, 1))
            b2nrow = singles.tile([1, D], dt.float32, tag="b2nr")
            nc.gpsimd.dma_start(out=b2nrow, in_=bcast_ap(b2n_d, 1))

            for b in range(NB):
                # ---- load this batch element ----
                XK = xtokp.tile([P, T, D], dt.float32, tag="xtok")
                for t in range(T):
                    if t < T - 1:
                        nc.sync.dma_start(
                            XK[:, t, :],
                            xtok[b, t * P:(t + 1) * P, :])
                    else:
                        nc.vector.memset(XK[:, t, :], 0.0)
                        nc.sync.dma_start(
                            XK[:TLAST, t, :],
                            xtok[b, t * P:t * P + TLAST, :])
                XT = xtp.tile([P, 2, N], dt.bfloat16, tag="xt")
                xTr = xT[b].rearrange("(s p) n -> p s n", p=P)
                for s in range(2):
                    for c in range(4):
                        c0 = c * 800
                        c1 = min(N, c0 + 800)
                        nc.sync.dma_start(XT[:, s, c0:c1], xTr[:, s, c0:c1])

                ST = stp.tile([P, T, NSLOT], dt.float32, tag="st")
                # pad lanes of the last tile get zero stats (written first,
                # then [0:TLAST] overwritten by the real ones)
                nc.vector.memset(ST[:, T - 1, :], 0.0)

                # ================= PHASE 1: per-token stats =================
                for t in range(T):
                    p_t = TLAST if t == T - 1 else P
                    xt_t = XK[:p_t, t, :]
                    mv6 = mvp.tile([P, 6], dt.float32, tag="mv6")
                    nc.vector.bn_stats(out=mv6[:p_t], in_=xt_t)
                    nc.vector.bn_aggr(out=ST[:p_t, t, MU:MU + 2], in_=mv6[:p_t])

                    py = psum.tile([P, 2 * D], dt.float32, tag="mm")
                    for s in range(2):
                        nc.tensor.matmul(
                            py[:p_t], XT[:, s, t * P:t * P + p_t],
                            s1_s[:, s, :], start=(s == 0), stop=(s == 1))
                    ps = psum.tile([P, 8], dt.float32, tag="sm")
                    for s in range(2):
                        nc.tensor.matmul(
                            ps[:p_t], XT[:, s, t * P:t * P + p_t],
                            s2_s[:, s, :], start=(s == 0), stop=(s == 1))

                    scr = work.tile([P, D], dt.float32, tag="w256")
                    nc.vector.scalar_tensor_tensor(
                        out=scr[:p_t], in0=xt_t, scalar=1.0,
                        in1=py[:p_t, 0:D], op0=ALU.mult, op1=ALU.mult,
                        accum_out=ST[:p_t, t, A1Q:A1Q + 1])
                    scr2 = work.tile([P, D], dt.float32, tag="w256")
                    nc.vector.scalar_tensor_tensor(
                        out=scr2[:p_t], in0=xt_t, scalar=1.0,
                        in1=py[:p_t, D:2 * D], op0=ALU.mult, op1=ALU.mult,
                        accum_out=ST[:p_t, t, A1K:A1K + 1])
                    nc.vector.tensor_copy(
                        out=ST[:p_t, t, XM1Q:XM1Q + 5], in_=ps[:p_t, 0:5])

                if phases == 1:
                    for t in range(T):
                        p_t = TLAST if t == T - 1 else P
                        nc.sync.dma_start(y[b, t * P:t * P + p_t, :],
                                          XK[:p_t, t, :])
                    continue

                # ============ PHASE 1.5a: batched stats math [128, T] ============
                def sl(i):
                    return ST[:, :, i]

                tA = t25.tile([P, T], dt.float32, tag="t25")
                nc.scalar.activation(out=tA, in_=sl(VAR), func=AF.Sqrt,
                                     bias=eps_t, scale=1.0)
                nc.vector.reciprocal(out=sl(R), in_=tA)
                rsq = t25.tile([P, T], dt.float32, tag="t25")
                nc.vector.tensor_mul(rsq, sl(R), sl(R))
                musq = t25.tile([P, T], dt.float32, tag="t25")
                nc.vector.tensor_mul(musq, sl(MU), sl(MU))

                def qk_norm(a1, xm1, xp, s1x, p1x, c0x, nzx, iqslot):
                    t1 = t25.tile([P, T], dt.float32, tag="t25")
                    nc.vector.tensor_mul(t1, sl(MU), sl(xm1))
                    t2 = t25.tile([P, T], dt.float32, tag="t25")
                    nc.vector.scalar_tensor_tensor(
                        out=t2, in0=t1, scalar=-2.0, in1=sl(a1),
                        op0=ALU.mult, op1=ALU.add)
                    t3 = t25.tile([P, T], dt.float32, tag="t25")
                    nc.vector.scalar_tensor_tensor(
                        out=t3, in0=musq, scalar=float(s1x), in1=t2,
                        op0=ALU.mult, op1=ALU.add)
                    sq = t25.tile([P, T], dt.float32, tag="t25")
                    nc.vector.tensor_mul(sq, t3, rsq)
                    if nzx:
                        t4 = t25.tile([P, T], dt.float32, tag="t25")
                        nc.vector.scalar_tensor_tensor(
                            out=t4, in0=sl(MU), scalar=float(-p1x), in1=sl(xp),
                            op0=ALU.mult, op1=ALU.add)
                        t5 = t25.tile([P, T], dt.float32, tag="t25")
                        nc.vector.tensor_mul(t5, t4, sl(R))
                        nc.vector.scalar_tensor_tensor(
                            out=sq, in0=t5, scalar=2.0, in1=sq,
                            op0=ALU.mult, op1=ALU.add)
                        nc.vector.tensor_scalar_add(sq, sq, float(c0x))
                    st = t25.tile([P, T], dt.float32, tag="t25")
                    nc.scalar.activation(out=st, in_=sq, func=AF.Sqrt,
                                         bias=0.0, scale=1.0)
                    nc.vector.tensor_scalar_max(st, st, L2_EPS)
                    nc.vector.reciprocal(out=sl(iqslot), in_=st)

                qk_norm(A1Q, XM1Q, XPQ, s1q, pq1, cq0, nz_pq, IQ)
                qk_norm(A1K, XM1K, XPK, s1k, pk1, ck0, nz_pk, IK)

                # u, a = u*iq
                tU = t25.tile([P, T], dt.float32, tag="t25")
                nc.vector.scalar_tensor_tensor(
                    out=tU, in0=sl(MU), scalar=float(-vg1), in1=sl(XVG),
                    op0=ALU.mult, op1=ALU.add)
                uu = t25.tile([P, T], dt.float32, tag="t25")
                nc.vector.tensor_mul(uu, tU, sl(R))
                if nz_u:
                    nc.vector.tensor_scalar_add(uu, uu, float(c0u))
                AA = t25.tile([P, T], dt.float32, tag="t25")
                nc.vector.tensor_mul(AA, uu, sl(IQ))

                # s' = ||a|| over all tokens -> 1/s' broadcast to [128,1]
                scrA = t25.tile([P, T], dt.float32, tag="t25")
                ssum = smallp.tile([P, 1], dt.float32, tag="sc1")
                nc.vector.scalar_tensor_tensor(
                    out=scrA, in0=AA, scalar=1.0, in1=AA,
                    op0=ALU.mult, op1=ALU.mult, accum_out=ssum)
                ps2 = psum.tile([1, 1], dt.float32, tag="sm")
                nc.tensor.matmul(ps2, ones_col_f, ssum)
                s_sc = smallp.tile([1, 1], dt.float32, tag="sc1")
                nc.scalar.activation(out=s_sc, in_=ps2, func=AF.Sqrt,
                                     bias=0.0, scale=1.0)
                pbc = psum.tile([P, 1], dt.float32, tag="sm")
                nc.tensor.matmul(pbc, ones_row_f, s_sc)
                sbc = smallp.tile([P, 1], dt.float32, tag="sc1")
                nc.vector.tensor_scalar_max(sbc, pbc, L2_EPS)
                inv_s = smallp.tile([P, 1], dt.float32, tag="sc1")
                nc.vector.reciprocal(out=inv_s, in_=sbc)

                # alpha = a*iq/s' ; WA = [alpha*r | alpha]; MO = [mu | 1]
                aiq = t25.tile([P, T], dt.float32, tag="t25")
                nc.vector.tensor_mul(aiq, AA, sl(IQ))
                AL = t25.tile([P, T], dt.float32, tag="t25")
                nc.vector.tensor_scalar_mul(AL, aiq, inv_s)
                WA = t25.tile([P, T, 2], dt.float32, tag="wa")
                nc.vector.tensor_mul(WA[:, :, 0], AL, sl(R))
                nc.vector.tensor_copy(out=WA[:, :, 1], in_=AL)
                MO = t25.tile([P, T, 2], dt.float32, tag="mo")
                nc.vector.tensor_copy(out=MO[:, :, 0], in_=sl(MU))
                nc.vector.memset(MO[:, :, 1], 1.0)

                # c1/c2 coefficient arrays
                nc.vector.tensor_mul(sl(C1Q), sl(IQ), sl(R))
                nc.vector.scalar_tensor_tensor(
                    out=sl(C2Q), in0=sl(C1Q), scalar=-1.0, in1=sl(MU),
                    op0=ALU.mult, op1=ALU.mult)
                nc.vector.tensor_mul(sl(C1K), sl(IK), sl(R))
                nc.vector.scalar_tensor_tensor(
                    out=sl(C2K), in0=sl(C1K), scalar=-1.0, in1=sl(MU),
                    op0=ALU.mult, op1=ALU.mult)

                # ============ PHASE 1.5b: t vector, G, Bk^T, bk1 ============
                pt = psum.tile([P, 4], dt.float32, tag="acc")
                for t in range(T):
                    first, last = (t == 0), (t == T - 1)
                    nc.tensor.matmul(pt[:, 0:1], XK[:, t, 0:P], WA[:, t, 0:1],
                                     start=first, stop=False,
                                     skip_group_check=True)
                    nc.tensor.matmul(pt[:, 1:2], XK[:, t, P:2 * P], WA[:, t, 0:1],
                                     start=False, stop=False,
                                     skip_group_check=True)
                    nc.tensor.matmul(pt[0:2, 2:4], MO[:, t, :], WA[:, t, :],
                                     start=False, stop=last,
                                     skip_group_check=True)
                sg_sb = smallp.tile([2, 2], dt.float32, tag="sg")
                nc.vector.tensor_copy(out=sg_sb, in_=pt[0:2, 2:4])
                pbb = psum.tile([P, 1], dt.float32, tag="sm")
                nc.tensor.matmul(pbb, ones_row_f, sg_sb[0:1, 0:1])
                sig1 = smallp.tile([P, 1], dt.float32, tag="sc1")
                nc.vector.tensor_copy(out=sig1, in_=pbb)
                t_sb = smallp.tile([P, 2], dt.float32, tag="tsb")
                nc.vector.tensor_scalar(
                    out=t_sb, in0=pt[:, 0:2], scalar1=sig1, scalar2=None,
                    op0=ALU.subtract)
                t_bf = smallp.tile([P, 2], dt.bfloat16, tag="tbf")
                nc.vector.tensor_copy(out=t_bf, in_=t_sb)

                pG = psum.tile([P, DI // P], dt.float32, tag="sm")
                for mc in range(DI // P):
                    for s in range(2):
                        nc.tensor.matmul(
                            pG[:, mc:mc + 1],
                            wqt_s[:, s, mc * P:(mc + 1) * P],
                            t_bf[:, s:s + 1],
                            start=(mc == 0 and s == 0),
                            stop=(mc == DI // P - 1 and s == 1),
                            skip_group_check=True)
                G_sb = smallp.tile([P, DI // P], dt.float32, tag="gsb")
                nc.vector.tensor_copy(out=G_sb, in_=pG)

                W1G = w1gp.tile([P, DI // P, D], dt.bfloat16, tag="w1g")
                for ks in range(DI // P):
                    nc.vector.tensor_scalar_mul(
                        W1G[:, ks, :], w1t_s[:, ks, :], G_sb[:, ks:ks + 1])

                pbk = psum.tile([P, 2, D], dt.float32, tag="mm")
                for mc2 in range(2):
                    for ks in range(DI // P):
                        nc.tensor.matmul(
                            pbk[:, mc2, :],
                            wke_s[:, ks, mc2 * P:(mc2 + 1) * P],
                            W1G[:, ks, :],
                            start=(mc2 == 0 and ks == 0),
                            stop=(mc2 == 1 and ks == DI // P - 1),
                            skip_group_check=True)
                BkT = bktp.tile([P, 2, D], dt.bfloat16, tag="bkt")
                nc.vector.tensor_copy(out=BkT, in_=pbk)

                pb1 = psum.tile([1, D], dt.float32, tag="sm")
                for s in range(2):
                    nc.tensor.matmul(pb1, ones_col_b, BkT[:, s, :],
                                     start=(s == 0), stop=(s == 1))
                bk1_row = smallp.tile([1, D], dt.bfloat16, tag="bk1r")
                nc.vector.tensor_copy(out=bk1_row, in_=pb1)
                pb1b = psum.tile([P, D], dt.float32, tag="sm")
                nc.tensor.matmul(pb1b, ones_row_b, bk1_row)
                bk1_t = bktp.tile([P, D], dt.float32, tag="bk1t")
                nc.vector.tensor_copy(out=bk1_t, in_=pb1b)

                # ================= PHASE 2: attn + residual =================
                for t in range(T):
                    p_t = TLAST if t == T - 1 else P
                    pa = psum.tile([P, 2 * D], dt.float32, tag="mm")
                    for s in range(2):
                        nc.tensor.matmul(
                            pa[:p_t, 0:D], XT[:, s, t * P:t * P + p_t],
                            bqt_s[:, s, :], start=(s == 0), stop=False,
                            skip_group_check=True)
                    for s in range(2):
                        nc.tensor.matmul(
                            pa[:p_t, D:2 * D], XT[:, s, t * P:t * P + p_t],
                            BkT[:, s, :], start=False, stop=(s == 1),
                            skip_group_check=True)
                    a1 = work.tile([P, D], dt.float32, tag="w256")
                    nc.vector.scalar_tensor_tensor(
                        out=a1[:p_t], in0=pa[:p_t, 0:D],
                        scalar=ST[:p_t, t, C1Q:C1Q + 1], in1=XK[:p_t, t, :],
                        op0=ALU.mult, op1=ALU.add)
                    a2 = work.tile([P, D], dt.float32, tag="w256")
                    nc.vector.scalar_tensor_tensor(
                        out=a2[:p_t], in0=pa[:p_t, D:2 * D],
                        scalar=ST[:p_t, t, C1K:C1K + 1], in1=a1[:p_t],
                        op0=ALU.mult, op1=ALU.add)
                    a3 = work.tile([P, D], dt.float32, tag="w256")
                    nc.vector.scalar_tensor_tensor(
                        out=a3[:p_t], in0=bq1_t[:p_t],
                        scalar=ST[:p_t, t, C2Q:C2Q + 1], in1=a2[:p_t],
                        op0=ALU.mult, op1=ALU.add)
                    nc.vector.scalar_tensor_tensor(
                        out=XK[:p_t, t, :], in0=bk1_t[:p_t],
                        scalar=ST[:p_t, t, C2K:C2K + 1], in1=a3[:p_t],
                        op0=ALU.mult, op1=ALU.add)

                if phases == 2:
                    for t in range(T):
                        p_t = TLAST if t == T - 1 else P
                        nc.sync.dma_start(y[b, t * P:t * P + p_t, :],
                                          XK[:p_t, t, :])
                    continue

                # ================= PHASE 3: MLP (feature-major) =================
                # batched LN2 stats (one ACT Sqrt per batch elem, so the ACT
                # table doesn't thrash between sqrt and gelu sets)
                ST2 = stp.tile([P, T, 2], dt.float32, tag="st2")
                nc.vector.memset(ST2[:, T - 1, :], 0.0)
                for t in range(T):
                    p_t = TLAST if t == T - 1 else P
                    mv6 = mvp.tile([P, 6], dt.float32, tag="mv6")
                    nc.vector.bn_stats(out=mv6[:p_t], in_=XK[:p_t, t, :])
                    nc.vector.bn_aggr(out=ST2[:p_t, t, 0:2], in_=mv6[:p_t])
                tV = t25.tile([P, T], dt.float32, tag="t25")
                nc.scalar.activation(out=tV, in_=ST2[:, :, 1], func=AF.Sqrt,
                                     bias=eps_t, scale=1.0)
                nc.vector.reciprocal(out=ST2[:, :, 1], in_=tV)

                for m in range(NMAC):
                    mtok = min(MACRO, N - m * MACRO)
                    ntt = (mtok + P - 1) // P
                    z2T = z2tp.tile([P, 2, MACRO], dt.bfloat16, tag="z2t")
                    z2_tiles = []
                    for ttx in range(ntt):
                        t = m * 4 + ttx
                        p_t = TLAST if t == T - 1 else P
                        z2 = z2p.tile([P, D], dt.bfloat16, tag="z2")
                        if p_t < P:
                            nc.vector.memset(z2, 0.0)
                        nc.vector.tensor_scalar(
                            out=z2[:p_t], in0=XK[:p_t, t, :],
                            scalar1=ST2[:p_t, t, 0:1], scalar2=ST2[:p_t, t, 1:2],
                            op0=ALU.subtract, op1=ALU.mult)
                        z2_tiles.append(z2)
                        for dc in range(2):
                            ptr = psum.tile([P, P], dt.bfloat16, tag="tr")
                            nc.tensor.transpose(
                                ptr[:, 0:p_t], z2[:p_t, dc * P:(dc + 1) * P],
                                ident[:p_t, :p_t])
                            nc.vector.tensor_copy(
                                out=z2T[:, dc, ttx * P:ttx * P + p_t],
                                in_=ptr[:, 0:p_t])

                    hdn = hdnp.tile([P, DH // P, MACRO], dt.bfloat16, tag="hdn")
                    for hc in range(DH // P):
                        ph = psum.tile([P, MACRO], dt.float32, tag="mm")
                        for s in range(2):
                            nc.tensor.matmul(
                                ph[:, 0:mtok],
                                m1t_s[:, s, hc * P:(hc + 1) * P],
                                z2T[:, s, 0:mtok],
                                start=(s == 0), stop=(s == 1))
                        nc.scalar.activation(
                            out=hdn[:, hc, 0:mtok], in_=ph[:, 0:mtok],
                            func=AF.Gelu, bias=b1e_s[:, hc:hc + 1], scale=1.0)

                    bodyT = bodyp.tile([P, 2, MACRO], dt.bfloat16, tag="body")
                    for dc in range(2):
                        pb = psum.tile([P, MACRO], dt.float32, tag="mm")
                        for ks in range(DH // P):
                            nc.tensor.matmul(
                                pb[:, 0:mtok],
                                m2t_s[:, ks, dc * P:(dc + 1) * P],
                                hdn[:, ks, 0:mtok],
                                start=(ks == 0), stop=(ks == DH // P - 1))
                        nc.scalar.activation(
                            out=bodyT[:, dc, 0:mtok], in_=pb[:, 0:mtok],
                            func=AF.Identity, bias=b2e_s[:, dc:dc + 1],
                            scale=1.0)

                    for ttx in range(ntt):
                        t = m * 4 + ttx
                        p_t = TLAST if t == T - 1 else P
                        ob = outp.tile([P, D], dt.float32, tag="out")
                        for dc in range(2):
                            ptb = psum.tile([P, P], dt.bfloat16, tag="tr")
                            nc.tensor.transpose(
                                ptb[:p_t, :],
                                bodyT[:, dc, ttx * P:ttx * P + p_t],
                                ident)
                            nc.vector.tensor_add(
                                out=ob[:p_t, dc * P:(dc + 1) * P],
                                in0=XK[:p_t, t, dc * P:(dc + 1) * P],
                                in1=ptb[:p_t, :])
                        if t == 0:
                            # CLS token bypasses the MLP: out0 = x2_0 + z2_0*g2+b2n
                            tcl = smallp.tile([1, D], dt.float32, tag="cls")
                            nc.vector.tensor_mul(tcl, z2_tiles[0][0:1, :], g2row)
                            nc.vector.tensor_add(tcl, tcl, b2nrow)
                            nc.vector.tensor_add(out=ob[0:1, :],
                                                 in0=XK[0:1, 0, :], in1=tcl)
                        nc.sync.dma_start(y[b, t * P:t * P + p_t, :],
                                          ob[:p_t, :])
    nc.finalize()
    return nc


_CACHE = {}


def _host_precompute(inputs):
    x = np.asarray(inputs["x"], F32)
    g1 = np.asarray(inputs["norm1_w"], F32)
    b1n = np.asarray(inputs["norm1_b"], F32)
    wq = np.asarray(inputs["wq"], F32)
    bq = np.asarray(inputs["bq"], F32)
    wk = np.asarray(inputs["wk"], F32)
    bk = np.asarray(inputs["bk"], F32)
    w_g = np.asarray(inputs["w_g"], F32)[:, 0]
    wp = np.asarray(inputs["wp"], F32)
    bp = np.asarray(inputs["bp"], F32)
    wf = np.asarray(inputs["wf"], F32)
    bf_ = np.asarray(inputs["bf"], F32)
    g2 = np.asarray(inputs["norm2_w"], F32)
    b2n = np.asarray(inputs["norm2_b"], F32)
    w1 = np.asarray(inputs["w1"], F32)
    b1 = np.asarray(inputs["b1"], F32)
    w2 = np.asarray(inputs["w2"], F32)
    b2 = np.asarray(inputs["b2"], F32)

    wq_eff = wq * g1[None, :]
    bq_eff = bq + wq @ b1n
    wk_eff = wk * g1[None, :]
    bk_eff = bk + wk @ b1n
    M_q = wq_eff.T @ wq_eff
    M_k = wk_eff.T @ wk_eff
    pq = wq_eff.T @ bq_eff
    pk = wk_eff.T @ bk_eff
    vg = wq_eff.T @ w_g
    W1 = wf @ wp
    Bq = wf @ wq_eff
    w1_eff = w1 * g2[None, :]
    b1_eff = b1 + w1 @ b2n

    consts = dict(
        s1q=float(M_q.sum()), s1k=float(M_k.sum()),
        pq1=float(pq.sum()), pk1=float(pk.sum()), vg1=float(vg.sum()),
        cq0=float(bq_eff @ bq_eff), ck0=float(bk_eff @ bk_eff),
        c0u=float(bq_eff @ w_g),
        nz_pq=bool(np.abs(bq_eff).max() > 0),
        nz_pk=bool(np.abs(bk_eff).max() > 0),
        nz_u=bool(abs(float(bq_eff @ w_g)) > 0),
    )
    assert np.abs(bk_eff).max() == 0 and np.abs(bq_eff).max() == 0 and \
        np.abs(wf @ bp + bf_).max() == 0, \
        "nonzero attention biases not supported by this build"

    weights = dict(
        s1=np.ascontiguousarray(np.concatenate([M_q, M_k], axis=1).astype(BF16)),
        s2=np.ascontiguousarray(np.stack(
            [M_q.sum(1), M_k.sum(1), pq, pk, vg,
             np.zeros(D, F32), np.zeros(D, F32), np.zeros(D, F32)],
            axis=1).astype(BF16)),
        bqt=np.ascontiguousarray(Bq.T.astype(BF16)),
        w1t=np.ascontiguousarray(W1.T.astype(BF16)),
        wqt=np.ascontiguousarray(wq_eff.T.astype(BF16)),
        wke=np.ascontiguousarray(wk_eff.astype(BF16)),
        m1t=np.ascontiguousarray(w1_eff.T.astype(BF16)),
        m2t=np.ascontiguousarray(w2.T.astype(BF16)),
        b1e=np.ascontiguousarray(b1_eff.astype(F32)),
        b2e=np.ascontiguousarray(b2.astype(F32)),
        bq1=np.ascontiguousarray(Bq.sum(1).astype(F32)),
        g2v=np.ascontiguousarray(g2.astype(F32)),
        b2n=np.ascontiguousarray(b2n.astype(F32)),
    )
    return x, consts, weights


def kernel(**inputs) -> np.ndarray:
    x, consts, weights = _host_precompute(inputs)
    xT = np.ascontiguousarray(x.transpose(0, 2, 1)).astype(BF16)

    key = tuple(sorted(consts.items()))
    if key not in _CACHE:
        _CACHE[key] = _build_program(consts)
    nc = _CACHE[key]

    in_maps = []
    for c in range(NC):
        m = dict(weights)
        m["xtok"] = np.ascontiguousarray(x[c * NB:(c + 1) * NB])
        m["xT"] = np.ascontiguousarray(xT[c * NB:(c + 1) * NB])
        in_maps.append(m)

    trace = bool(int(os.environ.get("KERNEL_TRACE", "0")))
    res = run_bass_kernel_spmd(nc, in_maps, core_ids=list(range(NC)),
                               trace=trace)
    if trace:
        kernel.last_results = res
    out = np.concatenate([res.results[c]["y"] for c in range(NC)], axis=0)
    return np.ascontiguousarray(out)


if __name__ == "__main__":
    d = dict(np.load("/root/problem/inputs.npz"))
    out = kernel(**d)
    exp = np.load("/root/problem/expected.npy")
    err = np.linalg.norm(out - exp) / np.linalg.norm(exp)
    print("rel l2 err:", err, "absmax diff:", np.abs(out - exp).max())
